# revision 74
# baseline (speedup 1.0000x reference)
"""Distributed causal self-attention kernel for 8 TRN2 NeuronCores.

Sharding: core c handles batch b=c//2 and head-half hf=c%2 (8 of 16 heads).
Per core: K0/Q0 projections stream against the x DMA; V for all 8 local
heads is computed once in natural [t, d] layout (no PE transposes); causal
flash attention in [tk, tq] layout with the triangular mask folded into the
scores PSUM via a constant matmul-add (exp of -30000 underflows to exactly
0), diagonal blocks column-trimmed to the live region; softmax denominators
via near-free [tq, 1]-output matmuls (the big et operand is the stationary
one); the next head's K/Q projections and RoPE are woven into the attention
emission to fill the exp-latency gaps, and the output projection (bf16,
chunked [128, 512]) is woven into head 7's attention. Outputs are bf16;
4 chunked 2-way ReduceScatters within each batch pair finish the partial
projections. Multi-wait instructions are legalized into single-wait NoOps
because this container's walrus rejects them.
"""

import sys

sys.path.insert(0, "/opt/trn_rl_repo")

import ml_dtypes
import numpy as np

import concourse.bass as bass
import concourse.mybir as mybir
from concourse.bass_utils import run_bass_kernel_spmd
from concourse.tile import TileContext

# Problem constants (hardcoded; kernel.py must be self-contained)
B, T, C = 4, 2048, 2048
H, D = 16, 128
HL = 8  # local heads per core
CLOC = HL * D  # 1024 local y features
ROPE_BASE = 500000
P = 128
NCT = C // P  # 16 contraction tiles
TQC = 512  # tq chunk
NTQ = T // TQC  # 4
NTK = T // P  # 16 tk tiles
SCALE = 1.0 / float(np.sqrt(D))
MASKNEG = -30000.0  # folded causal mask; exp(SCALE*(x+MASKNEG)) == 0 in f32

f32 = mybir.dt.float32
bf16 = mybir.dt.bfloat16
EXP = mybir.ActivationFunctionType.Exp


def build_nc(with_collective=True, debug=False):
    nc = bass.Bass(target_bir_lowering=False, num_devices=8)

    # per-core parameters (host pre-arranged layouts, all contiguous DMAs)
    xT = nc.declare_dram_parameter("xT", [P, NCT, T], bf16, isOutput=False)
    wq = nc.declare_dram_parameter("wq", [HL, P, NCT, P], bf16, isOutput=False)
    wk = nc.declare_dram_parameter("wk", [HL, P, NCT, P], bf16, isOutput=False)
    wv = nc.declare_dram_parameter("wv", [P, NCT, CLOC], bf16, isOutput=False)
    wp = nc.declare_dram_parameter("wp", [P, HL, C], bf16, isOutput=False)
    cost = nc.declare_dram_parameter("cost", [P, T], bf16, isOutput=False)
    sint = nc.declare_dram_parameter("sint", [P, T], bf16, isOutput=False)
    idnm = nc.declare_dram_parameter("idnm", [P, P], bf16, isOutput=False)
    onbm = nc.declare_dram_parameter("onbm", [P, P], bf16, isOutput=False)
    trim = nc.declare_dram_parameter("trim", [P, P], bf16, isOutput=False)
    selm = nc.declare_dram_parameter("selm", [P, 4 * P], bf16, isOutput=False)
    outp = nc.declare_dram_parameter("out", [CLOC, T], bf16, isOutput=True)
    dbgy = (
        nc.declare_dram_parameter("dbgy", [P, HL * T], bf16, isOutput=True)
        if debug
        else None
    )
    dbgd = (
        nc.declare_dram_parameter("dbgd", [P, 16 + 16 + 4 * TQC], f32, isOutput=True)
        if debug
        else None
    )

    poutT = nc.dram_tensor("poutT", [C, T], bf16)
    rs_out = nc.dram_tensor("rs_out", [CLOC, T], bf16)

    with TileContext(nc) as tc:
        with (
            tc.tile_pool(name="const", bufs=1) as cp,
            tc.tile_pool(name="xt", bufs=1) as xtp,
            tc.tile_pool(name="ybuf", bufs=1) as yp,
            tc.tile_pool(name="vbuf", bufs=1) as vbp,
            tc.tile_pool(name="rkq", bufs=4) as rkqp,
            tc.tile_pool(name="wqk", bufs=2) as wqkp,
            tc.tile_pool(name="rsw", bufs=2) as rsp,
        ):
            cos_sb = cp.tile([P, T], bf16, tag="cos")
            sin_sb = cp.tile([P, T], bf16, tag="sin")
            idn_sb = cp.tile([P, P], bf16, tag="idn")
            onb_sb = cp.tile([P, P], bf16, tag="onb")
            tri_sb = cp.tile([P, P], bf16, tag="tri")
            sel_sb = cp.tile([P, 4 * P], bf16, tag="sel")
            y2_sb = yp.tile([P, HL * T], bf16, tag="y2")
            v_sb = vbp.tile([P, NTK, CLOC], bf16, tag="v")

            # ---- startup DMAs: first K-weight tile and x chunk land fast ----
            wk_sb0 = wqkp.tile([P, NCT * P], bf16, tag="wqk", name="wk0")
            nc.sync.dma_start(out=wk_sb0[:, :P], in_=wk[0, :, 0, :])
            xt_sb = xtp.tile([P, NCT * T], bf16, tag="xt")
            nc.sync.dma_start(out=xt_sb[:, 0 : T // 4], in_=xT[:, 0, 0 : T // 4])
            nc.sync.dma_start(out=xt_sb[:, T // 4 : T], in_=xT[:, 0, T // 4 : T])
            nc.sync.dma_start(
                out=wk_sb0[:, P:], in_=wk[0, :, 1:, :].rearrange("p n d -> p (n d)")
            )
            nc.sync.dma_start(out=xt_sb[:, T : 2 * T], in_=xT[:, 1, :])
            nc.sync.dma_start(out=xt_sb[:, 2 * T : 3 * T], in_=xT[:, 2, :])
            wq_sb0 = wqkp.tile([P, NCT * P], bf16, tag="wqk", name="wq0")
            nc.sync.dma_start(out=wq_sb0[:], in_=wq[0].rearrange("p n d -> p (n d)"))
            for ci in range(3, NCT):
                nc.sync.dma_start(
                    out=xt_sb[:, ci * T : (ci + 1) * T], in_=xT[:, ci, :]
                )

            SWAP_MASK = [i ^ 1 for i in range(32)]

            def rope(src):
                # in-place: src = src*cos + swap(src)*sin (sin table signed;
                # the 1/sqrt(D) scale is folded into the exp activation);
                # swap = adjacent-partition exchange via DVE stream shuffle
                for cc2 in range(NTQ):
                    sl = slice(cc2 * TQC, (cc2 + 1) * TQC)
                    sw = rsp.tile([P, TQC], bf16, tag="rsw", name="swsb")
                    nc.vector.stream_shuffle(sw[:], src[:, sl], SWAP_MASK)
                    nc.vector.tensor_mul(sw[:], sw[:], sin_sb[:, sl])
                    nc.vector.tensor_mul(src[:, sl], src[:, sl], cos_sb[:, sl])
                    nc.vector.tensor_add(src[:, sl], src[:, sl], sw[:])

            # ---- head-0 K/Q projections streamed against the x DMA ----
            rk0 = rkqp.tile([P, T], bf16, tag="rkq", name="rk0")
            rq0 = rkqp.tile([P, T], bf16, tag="rkq", name="rq0")
            with tc.tile_pool(name="kq0", bufs=1, space="PSUM") as kq0p:
                kps = kq0p.tile([P, T], f32, tag="kps")
                qps = kq0p.tile([P, T], f32, tag="qps")

                def k0q0_mm(dst, w_sb, ci, tt):
                    nc.tensor.matmul(
                        dst[:, tt * TQC : (tt + 1) * TQC],
                        w_sb[:, ci * P : (ci + 1) * P],
                        xt_sb[:, ci * T + tt * TQC : ci * T + (tt + 1) * TQC],
                        start=(ci == 0),
                        stop=(ci == NCT - 1),
                        skip_group_check=True,
                    )

                # Q lags K by 3 ci-chunks: halves the early PE demand while
                # the x stream is still arriving
                for ci in range(NCT):
                    for tt in range(NTQ):
                        k0q0_mm(kps, wk_sb0, ci, tt)
                    if ci >= 3:
                        for tt in range(NTQ):
                            k0q0_mm(qps, wq_sb0, ci - 3, tt)
                for ci in range(NCT - 3, NCT):
                    for tt in range(NTQ):
                        k0q0_mm(qps, wq_sb0, ci, tt)
                # half-width evicts split across ACT and DVE so the serial
                # eviction chain (which gates the V phase) is halved
                for dst, src in ((rk0, kps), (rq0, qps)):
                    for tt in range(NTQ):
                        lo2 = tt * TQC
                        mid = lo2 + TQC // 2
                        hi = lo2 + TQC
                        nc.scalar.copy(dst[:, lo2:mid], src[:, lo2:mid])
                        nc.vector.tensor_copy(dst[:, mid:hi], src[:, mid:hi])

            def prefetch_kq(h):
                # weight DMAs + result tiles for head h's K/Q projections;
                # for h=1 this is emitted before the V pool scope so the DMA
                # doesn't inherit the end-of-V barrier
                wk_sb = wqkp.tile([P, NCT * P], bf16, tag="wqk", name="wkh")
                nc.sync.dma_start(
                    out=wk_sb[:], in_=wk[h].rearrange("p n d -> p (n d)")
                )
                wq_sb = wqkp.tile([P, NCT * P], bf16, tag="wqk", name="wqh")
                nc.sync.dma_start(
                    out=wq_sb[:], in_=wq[h].rearrange("p n d -> p (n d)")
                )
                rk = rkqp.tile([P, T], bf16, tag="rkq", name="rkh")
                rq = rkqp.tile([P, T], bf16, tag="rkq", name="rqh")
                return wk_sb, wq_sb, rk, rq

            pre1 = prefetch_kq(1)

            # ---- V for all 8 local heads, natural [t, d] layout ----
            with (
                tc.tile_pool(name="wv", bufs=1) as wvp,
                tc.tile_pool(name="vps", bufs=8, space="PSUM") as vpsp,
            ):
                wv_sb = wvp.tile([P, NCT * CLOC], bf16, tag="wv")
                for ci in range(4):
                    nc.sync.dma_start(
                        out=wv_sb[:, ci * CLOC : (ci + 1) * CLOC], in_=wv[:, ci, :]
                    )
                nc.sync.dma_start(out=cos_sb[:], in_=cost[:, :])
                nc.sync.dma_start(out=sin_sb[:], in_=sint[:, :])
                nc.sync.dma_start(out=idn_sb[:], in_=idnm[:, :])
                nc.sync.dma_start(out=onb_sb[:], in_=onbm[:, :])
                nc.sync.dma_start(out=tri_sb[:], in_=trim[:, :])
                nc.sync.dma_start(out=sel_sb[:], in_=selm[:, :])
                for ci in range(4, NCT):
                    nc.sync.dma_start(
                        out=wv_sb[:, ci * CLOC : (ci + 1) * CLOC], in_=wv[:, ci, :]
                    )
                # ragged waves: the tiny final wave lets the earlier waves'
                # eviction chains drain before attention needs the banks
                all_tiles = [(tt, hc) for tt in range(NTK) for hc in range(2)]
                waves = [all_tiles[0:8], all_tiles[8:16], all_tiles[16:24],
                         all_tiles[24:30], all_tiles[30:32]]
                for wave, tiles in enumerate(waves):
                    vt = [
                        vpsp.tile([P, TQC], f32, tag="vps", name="vt")
                        for _ in tiles
                    ]
                    for ci in range(NCT):
                        for s, (tt, hc) in enumerate(tiles):
                            nc.tensor.matmul(
                                vt[s][:],
                                xt_sb[:, ci * T + tt * P : ci * T + (tt + 1) * P],
                                wv_sb[:, ci * CLOC + hc * TQC : ci * CLOC + (hc + 1) * TQC],
                                start=(ci == 0),
                                stop=(ci == NCT - 1),
                            )
                    if wave == 0:
                        # rope(k0/q0) DVE work runs under wave-0's matmuls,
                        # queued before the wave evictions
                        rope(rk0)
                        rope(rq0)
                    for s, (tt, hc) in enumerate(tiles):
                        dst = v_sb[:, tt, hc * TQC : (hc + 1) * TQC]
                        if s % 2 == 0:
                            nc.scalar.copy(dst, vt[s][:])
                        else:
                            nc.vector.tensor_copy(dst, vt[s][:])

            # ---- steady state: attention(h) woven with K/Q(h+1) and proj ----
            with (
                tc.tile_pool(name="wp", bufs=1) as wpp,
                tc.tile_pool(name="et", bufs=3) as etp,
                tc.tile_pool(name="inv", bufs=2) as ivp,
                tc.tile_pool(name="pev", bufs=3) as pvp,
                tc.tile_pool(name="qk", bufs=2, space="PSUM") as qkp,
                tc.tile_pool(name="sc", bufs=2, space="PSUM") as scp,
                tc.tile_pool(name="psy", bufs=2, space="PSUM") as psyp,
                tc.tile_pool(name="sm", bufs=2, space="PSUM") as smp,
            ):
                wp_sb = wpp.tile([P, HL * C], bf16, tag="wp")

                pout_written = [False] * 16  # outp copy chunks emitted (sim path)

                def make_qk_tasks(h, pre):
                    """PE-work thunks building rk/rq for head h, finest grain."""
                    wk_sb, wq_sb, rk, rq = pre
                    # spread the proj-weight DMA across heads, after the K/Q
                    # weight loads so it never blocks the weave
                    nc.sync.dma_start(
                        out=wp_sb[:, (h - 1) * C : h * C], in_=wp[:, h - 1, :]
                    )
                    tasks = []
                    state = {}

                    def evict(dst, ps_key, tt):
                        def run():
                            sl = slice(tt * TQC, (tt + 1) * TQC)
                            nc.scalar.copy(dst[:, sl], state[ps_key][:])
                        return run

                    def rope_chunk(src, cc2):
                        def run():
                            sl = slice(cc2 * TQC, (cc2 + 1) * TQC)
                            sw = rsp.tile([P, TQC], bf16, tag="rsw", name="swsb")
                            nc.vector.stream_shuffle(sw[:], src[:, sl], SWAP_MASK)
                            nc.vector.tensor_mul(sw[:], sw[:], sin_sb[:, sl])
                            nc.vector.tensor_mul(src[:, sl], src[:, sl], cos_sb[:, sl])
                            nc.vector.tensor_add(src[:, sl], src[:, sl], sw[:])
                        return run

                    for dst, w_sb, key in ((rk, wk_sb, "k"), (rq, wq_sb, "q")):
                        for tt in range(NTQ):
                            def alloc(key=key, tt=tt):
                                state[(key, tt)] = qkp.tile(
                                    [P, TQC], f32, tag="qk", name="qkps"
                                )
                            tasks.append(alloc)
                            for ci in range(NCT):
                                def mm(dst=dst, w_sb=w_sb, key=key, tt=tt, ci=ci):
                                    nc.tensor.matmul(
                                        state[(key, tt)][:],
                                        w_sb[:, ci * P : (ci + 1) * P],
                                        xt_sb[:, ci * T + tt * TQC : ci * T + (tt + 1) * TQC],
                                        start=(ci == 0),
                                        stop=(ci == NCT - 1),
                                    )
                                tasks.append(mm)
                            tasks.append(evict(dst, (key, tt), tt))
                            tasks.append(rope_chunk(dst, tt))
                    return rk, rq, tasks

                def make_proj_tasks():
                    """Projection chunk-groups, tch-major so they unlock as
                    head 7's per-cc y2 chunks complete."""
                    nc.sync.dma_start(
                        out=wp_sb[:, (HL - 1) * C :], in_=wp[:, HL - 1, :]
                    )
                    tasks = []

                    def chunk(co, tch):
                        def run():
                            psj = qkp.tile([P, TQC], f32, tag="qk", name="psj")
                            for cin in range(HL):
                                nc.tensor.matmul(
                                    psj[:],
                                    wp_sb[:, cin * C + co * P : cin * C + (co + 1) * P],
                                    y2_sb[:, cin * T + tch * TQC : cin * T + (tch + 1) * TQC],
                                    start=(cin == 0),
                                    stop=(cin == HL - 1),
                                )
                            pev = pvp.tile([P, TQC], bf16, tag="pev", name="pev")
                            nc.vector.tensor_copy(pev[:], psj[:])
                            nc.sync.dma_start(
                                out=poutT[co * P : (co + 1) * P, tch * TQC : (tch + 1) * TQC],
                                in_=pev[:],
                            )
                        return run

                    def copy_out(g, tch):
                        def run():
                            nc.sync.dma_start(
                                out=outp[
                                    g * 256 : (g + 1) * 256,
                                    tch * TQC : (tch + 1) * TQC,
                                ],
                                in_=poutT[
                                    g * 256 : (g + 1) * 256,
                                    tch * TQC : (tch + 1) * TQC,
                                ],
                            )
                            pout_written[tch * 4 + g] = True
                        return run

                    for tch in range(NTQ):
                        # final tch: outp-feeding columns first so the copy
                        # DMAs drain under the trailing co 8..15 chunks
                        cos_ = (
                            list(range(NCT))
                            if tch < NTQ - 1
                            else [6, 7, 0, 1, 2, 3, 4, 5] + list(range(8, 16))
                        )
                        for co in cos_:
                            # gate: proj tch reads y2 written by head 7's
                            # cc=tch normalize — only pull after it is emitted
                            tasks.append((tch, chunk(co, tch)))
                            if not with_collective and co % 2 == 1 and co < HL:
                                tasks.append((tch, copy_out(co // 2, tch)))
                    return tasks

                def attention(h, rk, rq, fill):
                    """Causal attention for head h; pulls `fill` thunks into
                    the emission to keep the PE busy during exp latency.
                    Thunks may be (gate, fn): pulled only once this head's
                    cc=gate normalize has been emitted."""
                    fill = list(fill)
                    pos = [0]
                    gate_cc = [-1]  # highest cc whose normalize is emitted

                    def pull(n):
                        for _ in range(n):
                            if pos[0] >= len(fill):
                                return
                            t = fill[pos[0]]
                            if isinstance(t, tuple):
                                g, fn = t
                                if g > gate_cc[0]:
                                    return
                                t = fn
                            pos[0] += 1
                            t()

                    dbg_sb = (
                        ivp.tile([P, 16 + 16 + 4 * TQC], f32, tag="dbg", name="dbg")
                        if (debug and h == 0)
                        else None
                    )
                    for cc in range(NTQ):
                        njt = 4 * cc + 4
                        psy = psyp.tile([P, TQC], f32, tag="psy", name="psy")
                        pss4 = smp.tile(
                            [P, 4], f32, tag="sm", padded_shape=[P, TQC], name="pss4"
                        )
                        pending = []

                        def flush():
                            j, et, lo = pending.pop(0)
                            nc.tensor.matmul(
                                psy[:, lo:],
                                v_sb[:, j, h * P : (h + 1) * P],
                                et[:, lo:],
                                start=(j == 0),
                                stop=(j == njt - 1),
                                skip_group_check=True,
                            )
                            for s in range(lo // P, 4):
                                # start only on the very first sums matmul of
                                # this cc: PSUM start_tensor_calc marks the
                                # whole 2KB bank pending-zero, so a second
                                # start would poison the other columns'
                                # accumulation
                                nc.tensor.matmul(
                                    pss4[:, s : s + 1],
                                    et[:, s * P : (s + 1) * P],
                                    onb_sb[:, :1],
                                    start=(j == 0 and s == 0),
                                    stop=(j == 4 * cc + s),
                                    skip_group_check=True,
                                )

                        for j in range(njt):
                            rr = j - 4 * cc  # >= 0 on the block diagonal
                            lo = rr * P if rr >= 0 else 0
                            ps = scp.tile([P, TQC], f32, tag="sc", name="sc")
                            nc.tensor.matmul(
                                ps[:, lo:],
                                rk[:, j * P : (j + 1) * P],
                                rq[:, cc * TQC + lo : (cc + 1) * TQC],
                                start=True,
                                stop=(rr < 0),
                                skip_group_check=True,
                            )
                            if rr >= 0:  # fold causal mask into the diagonal tile
                                nc.tensor.matmul(
                                    ps[:, lo : lo + P],
                                    tri_sb[:],
                                    idn_sb[:],
                                    start=False,
                                    stop=True,
                                    skip_group_check=True,
                                )
                            et = etp.tile([P, TQC], bf16, tag="et", name="et")
                            nc.scalar.activation(
                                et[:, lo:], ps[:, lo:], EXP, scale=SCALE
                            )
                            if pending:
                                flush()
                            pending.append((j, et, lo))
                            pull(2 if j % 2 else 1)
                        flush()

                        # 1/rowsum broadcast: recip on [tq,4], transpose,
                        # replicate across partitions via tiny matmuls
                        inv4 = ivp.tile([P, 4], f32, tag="inv4", name="inv4")
                        nc.vector.reciprocal(inv4[:], pss4[:, :4])
                        inv4b = ivp.tile([P, 4], bf16, tag="inv4b", name="inv4b")
                        nc.vector.tensor_copy(inv4b[:], inv4[:])
                        psT = smp.tile(
                            [4, P], bf16, tag="sm", padded_shape=[4, 2 * TQC], name="psT"
                        )
                        nc.tensor.transpose(psT[:4, :], inv4b[:, :4], idn_sb[:])
                        invT = ivp.tile([4, P], bf16, tag="invT", name="invT")
                        nc.vector.tensor_copy(invT[:4, :], psT[:4, :])
                        psr = smp.tile([P, TQC], f32, tag="sm", name="psr")
                        for s in range(4):
                            # psr[:, s-block] = invT[s, :] broadcast across
                            # partitions: contraction over the 4 invT rows
                            # with a one-hot selector column block
                            nc.tensor.matmul(
                                psr[:, s * P : (s + 1) * P],
                                sel_sb[:4, s * P : (s + 1) * P],
                                invT[:4, :],
                                start=True,
                                stop=True,
                            )
                        invf = ivp.tile([P, TQC], bf16, tag="invf", name="invf")
                        nc.scalar.copy(invf[:], psr[:])
                        if debug and h == 0:
                            nc.vector.tensor_copy(
                                dbg_sb[:, 4 * cc : 4 * cc + 4], pss4[:, :4]
                            )
                            nc.vector.tensor_copy(
                                dbg_sb[:, 16 + 4 * cc : 16 + 4 * cc + 4], inv4[:, :4]
                            )
                            nc.vector.tensor_copy(
                                dbg_sb[:, 32 + cc * TQC : 32 + (cc + 1) * TQC], psr[:]
                            )
                            if cc == NTQ - 1:
                                nc.sync.dma_start(out=dbgd[:, :], in_=dbg_sb[:])
                        nc.vector.tensor_mul(
                            y2_sb[:, h * T + cc * TQC : h * T + (cc + 1) * TQC],
                            psy[:],
                            invf[:],
                        )
                        gate_cc[0] = cc
                        pull(3 + cc)
                    gate_cc[0] = NTQ
                    pull(10**9)  # drain whatever remains

                rk, rq = rk0, rq0
                pre = pre1
                for h in range(HL):
                    if h < HL - 1:
                        rk_n, rq_n, fill = make_qk_tasks(h + 1, pre)
                        if h + 2 < HL:
                            pre = prefetch_kq(h + 2)
                    else:
                        rk_n, rq_n, fill = None, None, make_proj_tasks()
                    attention(h, rk, rq, fill)
                    rk, rq = rk_n, rq_n
                if debug:
                    nc.sync.dma_start(out=dbgy[:, :], in_=y2_sb[:])

            # ---- reduce-scatter within batch pairs (real build only) ----
            if with_collective:
                for g in range(4):
                    nc.gpsimd.collective_compute(
                        "ReduceScatter",
                        mybir.AluOpType.add,
                        replica_groups=[[0, 1], [2, 3], [4, 5], [6, 7]],
                        ins=[poutT[g * 512 : (g + 1) * 512, :]],
                        outs=[rs_out[g * 256 : (g + 1) * 256, :]],
                    )
                    nc.sync.dma_start(
                        out=outp[g * 256 : (g + 1) * 256, :],
                        in_=rs_out[g * 256 : (g + 1) * 256, :],
                    )
            else:
                assert all(pout_written), "sim outp chunks missed"

    return nc


def _host_tables():
    inv_freq = 1.0 / (ROPE_BASE ** (np.arange(0, D, 2, dtype=np.float64) / D))
    pos = np.arange(T, dtype=np.float64)
    ang = pos[None, :] * inv_freq[:, None]  # [D/2, T]
    cos = np.cos(ang)
    sin = np.sin(ang)
    cost = np.empty((P, T), np.float32)
    sint = np.empty((P, T), np.float32)
    cost[0::2] = cos
    cost[1::2] = cos
    sint[0::2] = -sin
    sint[1::2] = sin
    # scores PSUM gets tri^T added on diagonal tiles: out[p, c] = MASKNEG
    # where p > c (tk > tq within the 128x128 diagonal sub-tile)
    trim_l = np.where(
        np.arange(P)[None, :] > np.arange(P)[:, None], np.float32(MASKNEG), 0.0
    )  # lhsT[k, i] = MASKNEG if i > k
    selm = np.zeros((P, 4 * P), np.float32)
    for s in range(4):
        selm[s, s * P : (s + 1) * P] = 1.0
    return (
        cost.astype(ml_dtypes.bfloat16),
        sint.astype(ml_dtypes.bfloat16),
        trim_l.astype(ml_dtypes.bfloat16),
        selm.astype(ml_dtypes.bfloat16),
    )


def _legalize_bir(bir_bytes):
    """Split multi-wait instructions into single-wait NoOps: this container's
    walrus codegen rejects >1 sync wait on f32/f32r matmuls and drains."""
    import json as _json

    bir = _json.loads(bir_bytes)
    n = 0
    for f in bir.get("functions", []):
        for b in f.get("blocks", []):
            new = []
            for inst in b["instructions"]:
                si = inst.get("sync_info") or {}
                waits = si.get("on_wait") or []
                if len(waits) > 1 and inst.get("engine"):
                    for w in waits[:-1]:
                        n += 1
                        new.append(
                            {
                                "name": f"{inst['name']}.lw{n}",
                                "opcode": "NoOp",
                                "engine": inst["engine"],
                                "ins": [],
                                "outs": [],
                                "sync_info": {"on_update": [], "on_wait": [w]},
                            }
                        )
                    si["on_wait"] = [waits[-1]]
                    inst["sync_info"] = si
                new.append(inst)
            b["instructions"] = new
    return _json.dumps(bir).encode()


def _install_compile_patch():
    import concourse.bass2jax as _b2j
    import concourse.bass_utils as _bu

    if getattr(_bu.compile_bir_kernel, "_legalized", False):
        return
    _orig = _bu.compile_bir_kernel

    def _patched(bir_json, tmpdir, neff_name="file.neff"):
        return _orig(_legalize_bir(bir_json), tmpdir, neff_name=neff_name)

    _patched._legalized = True
    _bu.compile_bir_kernel = _patched
    _b2j.compile_bir_kernel = _patched


_install_compile_patch()

_NC_CACHE = {}
_PROFILE = {"on": False, "exec_time_ns": None, "trace_dir": None, "times_ms": None}


def _run_timed(nc, in_maps, n_cores=8, iters=12):
    """Mirror bass2jax.run_bass_via_pjrt's multi-core path, but keep inputs
    on device and time repeated dispatches (no NTFF hook in this container)."""
    import time

    import jax
    from jax.experimental.shard_map import shard_map
    from jax.sharding import Mesh, NamedSharding, PartitionSpec

    from concourse import mybir as _mb
    from concourse.bass2jax import (
        _bass_exec_p,
        install_neuronx_cc_hook,
        partition_id_tensor,
    )

    install_neuronx_cc_hook()
    partition_name = nc.partition_id_tensor.name if nc.partition_id_tensor else None
    in_names, out_names, out_avals, zero_outs = [], [], [], []
    for alloc in nc.m.functions[0].allocations:
        if not isinstance(alloc, _mb.MemoryLocationSet):
            continue
        name = alloc.memorylocations[0].name
        if alloc.kind == "ExternalInput":
            if name != partition_name:
                in_names.append(name)
        elif alloc.kind == "ExternalOutput":
            out_names.append(name)
            shape = tuple(alloc.tensor_shape)
            dtype = _mb.dt.np(alloc.dtype)
            out_avals.append(jax.core.ShapedArray(shape, dtype))
            zero_outs.append(np.zeros(shape, dtype))
    n_params = len(in_names)
    all_in_names = list(in_names) + list(out_names)
    if partition_name is not None:
        all_in_names.append(partition_name)

    def _body(*args):
        operands = list(args)
        if partition_name is not None:
            operands.append(partition_id_tensor())
        outs = _bass_exec_p.bind(
            *operands,
            out_avals=tuple(out_avals),
            in_names=tuple(all_in_names),
            out_names=tuple(out_names),
            lowering_input_output_aliases=(),
            sim_require_finite=True,
            sim_require_nnan=True,
            nc=nc,
        )
        return tuple(outs)

    devices = jax.devices()[:n_cores]
    mesh = Mesh(np.asarray(devices), ("core",))
    spec = NamedSharding(mesh, PartitionSpec("core"))
    n_outs = len(out_avals)
    sharded = jax.jit(
        shard_map(
            _body,
            mesh=mesh,
            in_specs=(PartitionSpec("core"),) * (n_params + n_outs),
            out_specs=(PartitionSpec("core"),) * n_outs,
            check_rep=False,
        ),
        keep_unused=True,
    )
    concat_in = [
        jax.device_put(
            np.concatenate([np.asarray(in_maps[c][name]) for c in range(n_cores)], 0),
            spec,
        )
        for name in in_names
    ]
    concat_zeros = [
        jax.device_put(np.zeros((n_cores * z.shape[0], *z.shape[1:]), z.dtype), spec)
        for z in zero_outs
    ]
    out_arrs = sharded(*concat_in, *concat_zeros)  # warmup/compile
    jax.block_until_ready(out_arrs)
    times = []
    for _ in range(iters):
        t0 = time.perf_counter()
        r = sharded(*concat_in, *concat_zeros)
        jax.block_until_ready(r)
        times.append(time.perf_counter() - t0)
    _PROFILE["exec_time_ns"] = int(min(times) * 1e9)
    _PROFILE["times_ms"] = [t * 1e3 for t in times]
    results = [
        {
            name: np.asarray(out_arrs[i]).reshape(n_cores, *out_avals[i].shape)[c]
            for i, name in enumerate(out_names)
        }
        for c in range(n_cores)
    ]

    class _R:
        pass

    rr = _R()
    rr.results = results
    return rr


def kernel(x, Wqkv, Wproj):
    if "nc" not in _NC_CACHE:
        _NC_CACHE["nc"] = build_nc()
    nc = _NC_CACHE["nc"]

    x = np.asarray(x, np.float32)
    Wqkv = np.asarray(Wqkv, np.float32)
    Wproj = np.asarray(Wproj, np.float32)
    cost, sint, trim_l, selm = _host_tables()
    idnm = np.eye(P, dtype=ml_dtypes.bfloat16)
    onbm = np.ones((P, P), ml_dtypes.bfloat16)

    Wq, Wk, Wv = Wqkv[:, 0:C], Wqkv[:, C : 2 * C], Wqkv[:, 2 * C : 3 * C]

    def wtile(Wm, hf):  # [C, 1024] -> [HL, P, NCT, P] bf16
        Wl = Wm[:, hf * CLOC : (hf + 1) * CLOC]
        return np.ascontiguousarray(
            Wl.reshape(NCT, P, HL, P).transpose(2, 1, 0, 3).astype(ml_dtypes.bfloat16)
        )

    in_maps = []
    for c in range(8):
        b, hf = c // 2, c % 2
        xTc = np.ascontiguousarray(
            x[b].T.reshape(NCT, P, T).transpose(1, 0, 2).astype(ml_dtypes.bfloat16)
        )  # [P, NCT, T]
        wvc = np.ascontiguousarray(
            Wv[:, hf * CLOC : (hf + 1) * CLOC]
            .reshape(NCT, P, CLOC)
            .transpose(1, 0, 2)
            .astype(ml_dtypes.bfloat16)
        )  # [P, NCT, CLOC]
        wpc = np.ascontiguousarray(
            Wproj[hf * CLOC : (hf + 1) * CLOC, :]
            .reshape(HL, P, C)
            .transpose(1, 0, 2)
            .astype(ml_dtypes.bfloat16)
        )  # [P, HL, C]
        in_maps.append(
            {
                "xT": xTc,
                "wq": wtile(Wq, hf),
                "wk": wtile(Wk, hf),
                "wv": wvc,
                "wp": wpc,
                "cost": cost,
                "sint": sint,
                "idnm": idnm,
                "onbm": onbm,
                "trim": trim_l,
                "selm": selm,
            }
        )

    if _PROFILE.get("on"):
        res = _run_timed(nc, in_maps)
    else:
        res = run_bass_kernel_spmd(nc, in_maps, core_ids=list(range(8)))
    out = np.empty((B, T, C), np.float32)
    for c in range(8):
        b, hf = c // 2, c % 2
        r = np.asarray(res.results[c]["out"], dtype=np.float32)  # [1024, T]
        for g in range(4):
            cout0 = g * 512 + hf * 256
            out[b, :, cout0 : cout0 + 256] = r[g * 256 : (g + 1) * 256].T
    return out


if __name__ == "__main__":
    nc = build_nc()
    print("graph built ok:", len(nc.m.functions[0].allocations), "allocations")


# revision 76
# speedup vs baseline: 1.0318x; 1.0318x over previous
"""Distributed causal self-attention kernel for 8 TRN2 NeuronCores.

Sharding: core c handles batch b=c//2 and head-half hf=c%2 (8 of 16 heads).
Per core: K0/Q0 projections stream against the x DMA; V for all 8 local
heads is computed once in natural [t, d] layout (no PE transposes); causal
flash attention in [tk, tq] layout with the triangular mask folded into the
scores PSUM via a constant matmul-add (exp of -30000 underflows to exactly
0), diagonal blocks column-trimmed to the live region; softmax denominators
via near-free [tq, 1]-output matmuls (the big et operand is the stationary
one); the next head's K/Q projections and RoPE are woven into the attention
emission to fill the exp-latency gaps, and the output projection (bf16,
chunked [128, 512]) is woven into head 7's attention. Outputs are bf16;
4 chunked 2-way ReduceScatters within each batch pair finish the partial
projections. Multi-wait instructions are legalized into single-wait NoOps
because this container's walrus rejects them.
"""

import sys

sys.path.insert(0, "/opt/trn_rl_repo")

import ml_dtypes
import numpy as np

import concourse.bass as bass
import concourse.mybir as mybir
from concourse.bass_utils import run_bass_kernel_spmd
from concourse.tile import TileContext

# Problem constants (hardcoded; kernel.py must be self-contained)
B, T, C = 4, 2048, 2048
H, D = 16, 128
HL = 8  # local heads per core
CLOC = HL * D  # 1024 local y features
ROPE_BASE = 500000
P = 128
NCT = C // P  # 16 contraction tiles
TQC = 512  # tq chunk
NTQ = T // TQC  # 4
NTK = T // P  # 16 tk tiles
SCALE = 1.0 / float(np.sqrt(D))
MASKNEG = -30000.0  # folded causal mask; exp(SCALE*(x+MASKNEG)) == 0 in f32

f32 = mybir.dt.float32
bf16 = mybir.dt.bfloat16
EXP = mybir.ActivationFunctionType.Exp


def build_nc(with_collective=True, debug=False):
    nc = bass.Bass(target_bir_lowering=False, num_devices=8)

    # per-core parameters (host pre-arranged layouts, all contiguous DMAs)
    xT = nc.declare_dram_parameter("xT", [P, NCT, T], bf16, isOutput=False)
    wq = nc.declare_dram_parameter("wq", [HL, P, NCT, P], bf16, isOutput=False)
    wk = nc.declare_dram_parameter("wk", [HL, P, NCT, P], bf16, isOutput=False)
    wv = nc.declare_dram_parameter("wv", [P, NCT, CLOC], bf16, isOutput=False)
    wp = nc.declare_dram_parameter("wp", [P, HL, C], bf16, isOutput=False)
    cost = nc.declare_dram_parameter("cost", [P, T], bf16, isOutput=False)
    sint = nc.declare_dram_parameter("sint", [P, T], bf16, isOutput=False)
    idnm = nc.declare_dram_parameter("idnm", [P, P], bf16, isOutput=False)
    onbm = nc.declare_dram_parameter("onbm", [P, P], bf16, isOutput=False)
    trim = nc.declare_dram_parameter("trim", [P, P], bf16, isOutput=False)
    selm = nc.declare_dram_parameter("selm", [P, 4 * P], bf16, isOutput=False)
    outp = nc.declare_dram_parameter("out", [CLOC, T], bf16, isOutput=True)
    dbgy = (
        nc.declare_dram_parameter("dbgy", [P, HL * T], bf16, isOutput=True)
        if debug
        else None
    )
    dbgd = (
        nc.declare_dram_parameter("dbgd", [P, 16 + 16 + 4 * TQC], f32, isOutput=True)
        if debug
        else None
    )

    poutT = nc.dram_tensor("poutT", [C, T], bf16)
    rs_out = nc.dram_tensor("rs_out", [CLOC, T], bf16)

    with TileContext(nc) as tc:
        with (
            tc.tile_pool(name="const", bufs=1) as cp,
            tc.tile_pool(name="xt", bufs=1) as xtp,
            tc.tile_pool(name="ybuf", bufs=1) as yp,
            tc.tile_pool(name="vbuf", bufs=1) as vbp,
            tc.tile_pool(name="rkq", bufs=4) as rkqp,
            tc.tile_pool(name="wqk", bufs=2) as wqkp,
            tc.tile_pool(name="rsw", bufs=2) as rsp,
        ):
            cos_sb = cp.tile([P, T], bf16, tag="cos")
            sin_sb = cp.tile([P, T], bf16, tag="sin")
            idn_sb = cp.tile([P, P], bf16, tag="idn")
            onb_sb = cp.tile([P, P], bf16, tag="onb")
            tri_sb = cp.tile([P, P], bf16, tag="tri")
            sel_sb = cp.tile([P, 4 * P], bf16, tag="sel")
            y2_sb = yp.tile([P, HL * T], bf16, tag="y2")
            v_sb = vbp.tile([P, NTK, CLOC], bf16, tag="v")

            # ---- startup DMAs: first K-weight tile and x chunk land fast ----
            wk_sb0 = wqkp.tile([P, NCT * P], bf16, tag="wqk", name="wk0")
            nc.sync.dma_start(out=wk_sb0[:, :P], in_=wk[0, :, 0, :])
            xt_sb = xtp.tile([P, NCT * T], bf16, tag="xt")
            nc.sync.dma_start(out=xt_sb[:, 0 : T // 4], in_=xT[:, 0, 0 : T // 4])
            nc.sync.dma_start(out=xt_sb[:, T // 4 : T], in_=xT[:, 0, T // 4 : T])
            nc.sync.dma_start(
                out=wk_sb0[:, P:], in_=wk[0, :, 1:, :].rearrange("p n d -> p (n d)")
            )
            nc.sync.dma_start(out=xt_sb[:, T : 2 * T], in_=xT[:, 1, :])
            wq_sb0 = wqkp.tile([P, NCT * P], bf16, tag="wqk", name="wq0")
            nc.sync.dma_start(out=wq_sb0[:], in_=wq[0].rearrange("p n d -> p (n d)"))
            for ci in range(2, NCT):
                nc.sync.dma_start(
                    out=xt_sb[:, ci * T : (ci + 1) * T], in_=xT[:, ci, :]
                )

            SWAP_MASK = [i ^ 1 for i in range(32)]

            def rope(src):
                # in-place: src = src*cos + swap(src)*sin (sin table signed;
                # the 1/sqrt(D) scale is folded into the exp activation);
                # swap = adjacent-partition exchange via DVE stream shuffle
                for cc2 in range(NTQ):
                    sl = slice(cc2 * TQC, (cc2 + 1) * TQC)
                    sw = rsp.tile([P, TQC], bf16, tag="rsw", name="swsb")
                    nc.vector.stream_shuffle(sw[:], src[:, sl], SWAP_MASK)
                    nc.vector.tensor_mul(sw[:], sw[:], sin_sb[:, sl])
                    nc.vector.tensor_mul(src[:, sl], src[:, sl], cos_sb[:, sl])
                    nc.vector.tensor_add(src[:, sl], src[:, sl], sw[:])

            # ---- head-0 K/Q projections streamed against the x DMA ----
            rk0 = rkqp.tile([P, T], bf16, tag="rkq", name="rk0")
            rq0 = rkqp.tile([P, T], bf16, tag="rkq", name="rq0")
            with tc.tile_pool(name="kq0", bufs=1, space="PSUM") as kq0p:
                kps = kq0p.tile([P, T], f32, tag="kps")
                qps = kq0p.tile([P, T], f32, tag="qps")

                def k0q0_mm(dst, w_sb, ci, tt):
                    nc.tensor.matmul(
                        dst[:, tt * TQC : (tt + 1) * TQC],
                        w_sb[:, ci * P : (ci + 1) * P],
                        xt_sb[:, ci * T + tt * TQC : ci * T + (tt + 1) * TQC],
                        start=(ci == 0),
                        stop=(ci == NCT - 1),
                        skip_group_check=True,
                    )

                # Q lags K by 3 ci-chunks: halves the early PE demand while
                # the x stream is still arriving
                for ci in range(NCT):
                    for tt in range(NTQ):
                        k0q0_mm(kps, wk_sb0, ci, tt)
                    if ci >= 3:
                        for tt in range(NTQ):
                            k0q0_mm(qps, wq_sb0, ci - 3, tt)
                for ci in range(NCT - 3, NCT):
                    for tt in range(NTQ):
                        k0q0_mm(qps, wq_sb0, ci, tt)
                for tt in range(NTQ):
                    sl = slice(tt * TQC, (tt + 1) * TQC)
                    if tt % 2 == 0:
                        nc.scalar.copy(rk0[:, sl], kps[:, sl])
                    else:
                        nc.vector.tensor_copy(rk0[:, sl], kps[:, sl])
                for tt in range(NTQ):
                    sl = slice(tt * TQC, (tt + 1) * TQC)
                    if tt % 2 == 0:
                        nc.vector.tensor_copy(rq0[:, sl], qps[:, sl])
                    else:
                        nc.scalar.copy(rq0[:, sl], qps[:, sl])

            def prefetch_kq(h):
                # weight DMAs + result tiles for head h's K/Q projections;
                # for h=1 this is emitted before the V pool scope so the DMA
                # doesn't inherit the end-of-V barrier
                wk_sb = wqkp.tile([P, NCT * P], bf16, tag="wqk", name="wkh")
                nc.sync.dma_start(
                    out=wk_sb[:], in_=wk[h].rearrange("p n d -> p (n d)")
                )
                wq_sb = wqkp.tile([P, NCT * P], bf16, tag="wqk", name="wqh")
                nc.sync.dma_start(
                    out=wq_sb[:], in_=wq[h].rearrange("p n d -> p (n d)")
                )
                rk = rkqp.tile([P, T], bf16, tag="rkq", name="rkh")
                rq = rkqp.tile([P, T], bf16, tag="rkq", name="rqh")
                return wk_sb, wq_sb, rk, rq

            pre1 = prefetch_kq(1)

            # ---- V for all 8 local heads, natural [t, d] layout ----
            with (
                tc.tile_pool(name="wv", bufs=1) as wvp,
                tc.tile_pool(name="vps", bufs=8, space="PSUM") as vpsp,
            ):
                wv_sb = wvp.tile([P, NCT * CLOC], bf16, tag="wv")
                for ci in range(4):
                    nc.sync.dma_start(
                        out=wv_sb[:, ci * CLOC : (ci + 1) * CLOC], in_=wv[:, ci, :]
                    )
                nc.sync.dma_start(out=cos_sb[:], in_=cost[:, :])
                nc.sync.dma_start(out=sin_sb[:], in_=sint[:, :])
                nc.sync.dma_start(out=idn_sb[:], in_=idnm[:, :])
                nc.sync.dma_start(out=onb_sb[:], in_=onbm[:, :])
                nc.sync.dma_start(out=tri_sb[:], in_=trim[:, :])
                nc.sync.dma_start(out=sel_sb[:], in_=selm[:, :])
                for ci in range(4, NCT):
                    nc.sync.dma_start(
                        out=wv_sb[:, ci * CLOC : (ci + 1) * CLOC], in_=wv[:, ci, :]
                    )
                # ragged waves: the tiny final wave lets the earlier waves'
                # eviction chains drain before attention needs the banks
                all_tiles = [(tt, hc) for tt in range(NTK) for hc in range(2)]
                waves = [all_tiles[0:8], all_tiles[8:16], all_tiles[16:24],
                         all_tiles[24:30], all_tiles[30:32]]
                for wave, tiles in enumerate(waves):
                    vt = [
                        vpsp.tile([P, TQC], f32, tag="vps", name="vt")
                        for _ in tiles
                    ]
                    for ci in range(NCT):
                        for s, (tt, hc) in enumerate(tiles):
                            nc.tensor.matmul(
                                vt[s][:],
                                xt_sb[:, ci * T + tt * P : ci * T + (tt + 1) * P],
                                wv_sb[:, ci * CLOC + hc * TQC : ci * CLOC + (hc + 1) * TQC],
                                start=(ci == 0),
                                stop=(ci == NCT - 1),
                            )
                    if wave == 0:
                        # rope(k0/q0) DVE work runs under wave-0's matmuls,
                        # queued before the wave evictions
                        rope(rk0)
                        rope(rq0)
                    for s, (tt, hc) in enumerate(tiles):
                        dst = v_sb[:, tt, hc * TQC : (hc + 1) * TQC]
                        if s % 2 == 0:
                            nc.scalar.copy(dst, vt[s][:])
                        else:
                            nc.vector.tensor_copy(dst, vt[s][:])

            # ---- steady state: attention(h) woven with K/Q(h+1) and proj ----
            with (
                tc.tile_pool(name="wp", bufs=1) as wpp,
                tc.tile_pool(name="et", bufs=3) as etp,
                tc.tile_pool(name="inv", bufs=2) as ivp,
                tc.tile_pool(name="pev", bufs=3) as pvp,
                tc.tile_pool(name="qk", bufs=2, space="PSUM") as qkp,
                tc.tile_pool(name="sc", bufs=2, space="PSUM") as scp,
                tc.tile_pool(name="psy", bufs=2, space="PSUM") as psyp,
                tc.tile_pool(name="sm", bufs=2, space="PSUM") as smp,
            ):
                wp_sb = wpp.tile([P, HL * C], bf16, tag="wp")

                pout_written = [False] * 16  # outp copy chunks emitted (sim path)

                def make_qk_tasks(h, pre):
                    """PE-work thunks building rk/rq for head h, finest grain."""
                    wk_sb, wq_sb, rk, rq = pre
                    # spread the proj-weight DMA across heads, after the K/Q
                    # weight loads so it never blocks the weave
                    nc.sync.dma_start(
                        out=wp_sb[:, (h - 1) * C : h * C], in_=wp[:, h - 1, :]
                    )
                    tasks = []
                    state = {}

                    def evict(dst, ps_key, tt):
                        def run():
                            sl = slice(tt * TQC, (tt + 1) * TQC)
                            nc.scalar.copy(dst[:, sl], state[ps_key][:])
                        return run

                    def rope_chunk(src, cc2):
                        def run():
                            sl = slice(cc2 * TQC, (cc2 + 1) * TQC)
                            sw = rsp.tile([P, TQC], bf16, tag="rsw", name="swsb")
                            nc.vector.stream_shuffle(sw[:], src[:, sl], SWAP_MASK)
                            nc.vector.tensor_mul(sw[:], sw[:], sin_sb[:, sl])
                            nc.vector.tensor_mul(src[:, sl], src[:, sl], cos_sb[:, sl])
                            nc.vector.tensor_add(src[:, sl], src[:, sl], sw[:])
                        return run

                    for dst, w_sb, key in ((rk, wk_sb, "k"), (rq, wq_sb, "q")):
                        for tt in range(NTQ):
                            def alloc(key=key, tt=tt):
                                state[(key, tt)] = qkp.tile(
                                    [P, TQC], f32, tag="qk", name="qkps"
                                )
                            tasks.append(alloc)
                            for ci in range(NCT):
                                def mm(dst=dst, w_sb=w_sb, key=key, tt=tt, ci=ci):
                                    nc.tensor.matmul(
                                        state[(key, tt)][:],
                                        w_sb[:, ci * P : (ci + 1) * P],
                                        xt_sb[:, ci * T + tt * TQC : ci * T + (tt + 1) * TQC],
                                        start=(ci == 0),
                                        stop=(ci == NCT - 1),
                                    )
                                tasks.append(mm)
                            tasks.append(evict(dst, (key, tt), tt))
                            tasks.append(rope_chunk(dst, tt))
                    return rk, rq, tasks

                def make_proj_tasks():
                    """Projection chunk-groups, tch-major so they unlock as
                    head 7's per-cc y2 chunks complete."""
                    nc.sync.dma_start(
                        out=wp_sb[:, (HL - 1) * C :], in_=wp[:, HL - 1, :]
                    )
                    tasks = []

                    def chunk(co, tch):
                        def run():
                            psj = qkp.tile([P, TQC], f32, tag="qk", name="psj")
                            for cin in range(HL):
                                nc.tensor.matmul(
                                    psj[:],
                                    wp_sb[:, cin * C + co * P : cin * C + (co + 1) * P],
                                    y2_sb[:, cin * T + tch * TQC : cin * T + (tch + 1) * TQC],
                                    start=(cin == 0),
                                    stop=(cin == HL - 1),
                                )
                            pev = pvp.tile([P, TQC], bf16, tag="pev", name="pev")
                            nc.vector.tensor_copy(pev[:], psj[:])
                            nc.sync.dma_start(
                                out=poutT[co * P : (co + 1) * P, tch * TQC : (tch + 1) * TQC],
                                in_=pev[:],
                            )
                        return run

                    def copy_out(g, tch):
                        def run():
                            nc.sync.dma_start(
                                out=outp[
                                    g * 256 : (g + 1) * 256,
                                    tch * TQC : (tch + 1) * TQC,
                                ],
                                in_=poutT[
                                    g * 256 : (g + 1) * 256,
                                    tch * TQC : (tch + 1) * TQC,
                                ],
                            )
                            pout_written[tch * 4 + g] = True
                        return run

                    for tch in range(NTQ):
                        # final tch: outp-feeding columns first so the copy
                        # DMAs drain under the trailing co 8..15 chunks
                        cos_ = (
                            list(range(NCT))
                            if tch < NTQ - 1
                            else [6, 7, 0, 1, 2, 3, 4, 5] + list(range(8, 16))
                        )
                        for co in cos_:
                            # gate: proj tch reads y2 written by head 7's
                            # cc=tch normalize — only pull after it is emitted
                            tasks.append((tch, chunk(co, tch)))
                            if not with_collective and co % 2 == 1 and co < HL:
                                tasks.append((tch, copy_out(co // 2, tch)))
                    return tasks

                def attention(h, rk, rq, fill):
                    """Causal attention for head h; pulls `fill` thunks into
                    the emission to keep the PE busy during exp latency.
                    Thunks may be (gate, fn): pulled only once this head's
                    cc=gate normalize has been emitted."""
                    fill = list(fill)
                    pos = [0]
                    gate_cc = [-1]  # highest cc whose normalize is emitted

                    def pull(n):
                        for _ in range(n):
                            if pos[0] >= len(fill):
                                return
                            t = fill[pos[0]]
                            if isinstance(t, tuple):
                                g, fn = t
                                if g > gate_cc[0]:
                                    return
                                t = fn
                            pos[0] += 1
                            t()

                    dbg_sb = (
                        ivp.tile([P, 16 + 16 + 4 * TQC], f32, tag="dbg", name="dbg")
                        if (debug and h == 0)
                        else None
                    )
                    for cc in range(NTQ):
                        njt = 4 * cc + 4
                        psy = psyp.tile([P, TQC], f32, tag="psy", name="psy")
                        pss4 = smp.tile(
                            [P, 4], f32, tag="sm", padded_shape=[P, TQC], name="pss4"
                        )
                        pending = []

                        def flush():
                            j, et, lo = pending.pop(0)
                            nc.tensor.matmul(
                                psy[:, lo:],
                                v_sb[:, j, h * P : (h + 1) * P],
                                et[:, lo:],
                                start=(j == 0),
                                stop=(j == njt - 1),
                                skip_group_check=True,
                            )
                            for s in range(lo // P, 4):
                                # start only on the very first sums matmul of
                                # this cc: PSUM start_tensor_calc marks the
                                # whole 2KB bank pending-zero, so a second
                                # start would poison the other columns'
                                # accumulation
                                nc.tensor.matmul(
                                    pss4[:, s : s + 1],
                                    et[:, s * P : (s + 1) * P],
                                    onb_sb[:, :1],
                                    start=(j == 0 and s == 0),
                                    stop=(j == 4 * cc + s),
                                    skip_group_check=True,
                                )

                        for j in range(njt):
                            rr = j - 4 * cc  # >= 0 on the block diagonal
                            lo = rr * P if rr >= 0 else 0
                            ps = scp.tile([P, TQC], f32, tag="sc", name="sc")
                            nc.tensor.matmul(
                                ps[:, lo:],
                                rk[:, j * P : (j + 1) * P],
                                rq[:, cc * TQC + lo : (cc + 1) * TQC],
                                start=True,
                                stop=(rr < 0),
                                skip_group_check=True,
                            )
                            if rr >= 0:  # fold causal mask into the diagonal tile
                                nc.tensor.matmul(
                                    ps[:, lo : lo + P],
                                    tri_sb[:],
                                    idn_sb[:],
                                    start=False,
                                    stop=True,
                                    skip_group_check=True,
                                )
                            et = etp.tile([P, TQC], bf16, tag="et", name="et")
                            nc.scalar.activation(
                                et[:, lo:], ps[:, lo:], EXP, scale=SCALE
                            )
                            if pending:
                                flush()
                            pending.append((j, et, lo))
                            pull(2 if j % 2 else 1)
                        flush()

                        # 1/rowsum broadcast: recip on [tq,4], transpose,
                        # replicate across partitions via tiny matmuls
                        inv4 = ivp.tile([P, 4], f32, tag="inv4", name="inv4")
                        nc.vector.reciprocal(inv4[:], pss4[:, :4])
                        inv4b = ivp.tile([P, 4], bf16, tag="inv4b", name="inv4b")
                        nc.vector.tensor_copy(inv4b[:], inv4[:])
                        psT = smp.tile(
                            [4, P], bf16, tag="sm", padded_shape=[4, 2 * TQC], name="psT"
                        )
                        nc.tensor.transpose(psT[:4, :], inv4b[:, :4], idn_sb[:])
                        invT = ivp.tile([4, P], bf16, tag="invT", name="invT")
                        nc.vector.tensor_copy(invT[:4, :], psT[:4, :])
                        psr = smp.tile([P, TQC], f32, tag="sm", name="psr")
                        for s in range(4):
                            # psr[:, s-block] = invT[s, :] broadcast across
                            # partitions: contraction over the 4 invT rows
                            # with a one-hot selector column block
                            nc.tensor.matmul(
                                psr[:, s * P : (s + 1) * P],
                                sel_sb[:4, s * P : (s + 1) * P],
                                invT[:4, :],
                                start=True,
                                stop=True,
                            )
                        invf = ivp.tile([P, TQC], bf16, tag="invf", name="invf")
                        nc.scalar.copy(invf[:], psr[:])
                        if debug and h == 0:
                            nc.vector.tensor_copy(
                                dbg_sb[:, 4 * cc : 4 * cc + 4], pss4[:, :4]
                            )
                            nc.vector.tensor_copy(
                                dbg_sb[:, 16 + 4 * cc : 16 + 4 * cc + 4], inv4[:, :4]
                            )
                            nc.vector.tensor_copy(
                                dbg_sb[:, 32 + cc * TQC : 32 + (cc + 1) * TQC], psr[:]
                            )
                            if cc == NTQ - 1:
                                nc.sync.dma_start(out=dbgd[:, :], in_=dbg_sb[:])
                        nc.vector.tensor_mul(
                            y2_sb[:, h * T + cc * TQC : h * T + (cc + 1) * TQC],
                            psy[:],
                            invf[:],
                        )
                        gate_cc[0] = cc
                        pull(3 + cc)
                    gate_cc[0] = NTQ
                    pull(10**9)  # drain whatever remains

                rk, rq = rk0, rq0
                pre = pre1
                for h in range(HL):
                    if h < HL - 1:
                        rk_n, rq_n, fill = make_qk_tasks(h + 1, pre)
                        if h + 2 < HL:
                            pre = prefetch_kq(h + 2)
                    else:
                        rk_n, rq_n, fill = None, None, make_proj_tasks()
                    attention(h, rk, rq, fill)
                    rk, rq = rk_n, rq_n
                if debug:
                    nc.sync.dma_start(out=dbgy[:, :], in_=y2_sb[:])

            # ---- reduce-scatter within batch pairs (real build only) ----
            if with_collective:
                for g in range(4):
                    nc.gpsimd.collective_compute(
                        "ReduceScatter",
                        mybir.AluOpType.add,
                        replica_groups=[[0, 1], [2, 3], [4, 5], [6, 7]],
                        ins=[poutT[g * 512 : (g + 1) * 512, :]],
                        outs=[rs_out[g * 256 : (g + 1) * 256, :]],
                    )
                    nc.sync.dma_start(
                        out=outp[g * 256 : (g + 1) * 256, :],
                        in_=rs_out[g * 256 : (g + 1) * 256, :],
                    )
            else:
                assert all(pout_written), "sim outp chunks missed"

    return nc


def _host_tables():
    inv_freq = 1.0 / (ROPE_BASE ** (np.arange(0, D, 2, dtype=np.float64) / D))
    pos = np.arange(T, dtype=np.float64)
    ang = pos[None, :] * inv_freq[:, None]  # [D/2, T]
    cos = np.cos(ang)
    sin = np.sin(ang)
    cost = np.empty((P, T), np.float32)
    sint = np.empty((P, T), np.float32)
    cost[0::2] = cos
    cost[1::2] = cos
    sint[0::2] = -sin
    sint[1::2] = sin
    # scores PSUM gets tri^T added on diagonal tiles: out[p, c] = MASKNEG
    # where p > c (tk > tq within the 128x128 diagonal sub-tile)
    trim_l = np.where(
        np.arange(P)[None, :] > np.arange(P)[:, None], np.float32(MASKNEG), 0.0
    )  # lhsT[k, i] = MASKNEG if i > k
    selm = np.zeros((P, 4 * P), np.float32)
    for s in range(4):
        selm[s, s * P : (s + 1) * P] = 1.0
    return (
        cost.astype(ml_dtypes.bfloat16),
        sint.astype(ml_dtypes.bfloat16),
        trim_l.astype(ml_dtypes.bfloat16),
        selm.astype(ml_dtypes.bfloat16),
    )


def _legalize_bir(bir_bytes):
    """Split multi-wait instructions into single-wait NoOps: this container's
    walrus codegen rejects >1 sync wait on f32/f32r matmuls and drains."""
    import json as _json

    bir = _json.loads(bir_bytes)
    n = 0
    for f in bir.get("functions", []):
        for b in f.get("blocks", []):
            new = []
            for inst in b["instructions"]:
                si = inst.get("sync_info") or {}
                waits = si.get("on_wait") or []
                if len(waits) > 1 and inst.get("engine"):
                    for w in waits[:-1]:
                        n += 1
                        new.append(
                            {
                                "name": f"{inst['name']}.lw{n}",
                                "opcode": "NoOp",
                                "engine": inst["engine"],
                                "ins": [],
                                "outs": [],
                                "sync_info": {"on_update": [], "on_wait": [w]},
                            }
                        )
                    si["on_wait"] = [waits[-1]]
                    inst["sync_info"] = si
                new.append(inst)
            b["instructions"] = new
    return _json.dumps(bir).encode()


def _install_compile_patch():
    import concourse.bass2jax as _b2j
    import concourse.bass_utils as _bu

    if getattr(_bu.compile_bir_kernel, "_legalized", False):
        return
    _orig = _bu.compile_bir_kernel

    def _patched(bir_json, tmpdir, neff_name="file.neff"):
        return _orig(_legalize_bir(bir_json), tmpdir, neff_name=neff_name)

    _patched._legalized = True
    _bu.compile_bir_kernel = _patched
    _b2j.compile_bir_kernel = _patched


_install_compile_patch()

_NC_CACHE = {}
_PROFILE = {"on": False, "exec_time_ns": None, "trace_dir": None, "times_ms": None}


def _run_timed(nc, in_maps, n_cores=8, iters=12):
    """Mirror bass2jax.run_bass_via_pjrt's multi-core path, but keep inputs
    on device and time repeated dispatches (no NTFF hook in this container)."""
    import time

    import jax
    from jax.experimental.shard_map import shard_map
    from jax.sharding import Mesh, NamedSharding, PartitionSpec

    from concourse import mybir as _mb
    from concourse.bass2jax import (
        _bass_exec_p,
        install_neuronx_cc_hook,
        partition_id_tensor,
    )

    install_neuronx_cc_hook()
    partition_name = nc.partition_id_tensor.name if nc.partition_id_tensor else None
    in_names, out_names, out_avals, zero_outs = [], [], [], []
    for alloc in nc.m.functions[0].allocations:
        if not isinstance(alloc, _mb.MemoryLocationSet):
            continue
        name = alloc.memorylocations[0].name
        if alloc.kind == "ExternalInput":
            if name != partition_name:
                in_names.append(name)
        elif alloc.kind == "ExternalOutput":
            out_names.append(name)
            shape = tuple(alloc.tensor_shape)
            dtype = _mb.dt.np(alloc.dtype)
            out_avals.append(jax.core.ShapedArray(shape, dtype))
            zero_outs.append(np.zeros(shape, dtype))
    n_params = len(in_names)
    all_in_names = list(in_names) + list(out_names)
    if partition_name is not None:
        all_in_names.append(partition_name)

    def _body(*args):
        operands = list(args)
        if partition_name is not None:
            operands.append(partition_id_tensor())
        outs = _bass_exec_p.bind(
            *operands,
            out_avals=tuple(out_avals),
            in_names=tuple(all_in_names),
            out_names=tuple(out_names),
            lowering_input_output_aliases=(),
            sim_require_finite=True,
            sim_require_nnan=True,
            nc=nc,
        )
        return tuple(outs)

    devices = jax.devices()[:n_cores]
    mesh = Mesh(np.asarray(devices), ("core",))
    spec = NamedSharding(mesh, PartitionSpec("core"))
    n_outs = len(out_avals)
    sharded = jax.jit(
        shard_map(
            _body,
            mesh=mesh,
            in_specs=(PartitionSpec("core"),) * (n_params + n_outs),
            out_specs=(PartitionSpec("core"),) * n_outs,
            check_rep=False,
        ),
        keep_unused=True,
    )
    concat_in = [
        jax.device_put(
            np.concatenate([np.asarray(in_maps[c][name]) for c in range(n_cores)], 0),
            spec,
        )
        for name in in_names
    ]
    concat_zeros = [
        jax.device_put(np.zeros((n_cores * z.shape[0], *z.shape[1:]), z.dtype), spec)
        for z in zero_outs
    ]
    out_arrs = sharded(*concat_in, *concat_zeros)  # warmup/compile
    jax.block_until_ready(out_arrs)
    times = []
    for _ in range(iters):
        t0 = time.perf_counter()
        r = sharded(*concat_in, *concat_zeros)
        jax.block_until_ready(r)
        times.append(time.perf_counter() - t0)
    _PROFILE["exec_time_ns"] = int(min(times) * 1e9)
    _PROFILE["times_ms"] = [t * 1e3 for t in times]
    results = [
        {
            name: np.asarray(out_arrs[i]).reshape(n_cores, *out_avals[i].shape)[c]
            for i, name in enumerate(out_names)
        }
        for c in range(n_cores)
    ]

    class _R:
        pass

    rr = _R()
    rr.results = results
    return rr


def kernel(x, Wqkv, Wproj):
    if "nc" not in _NC_CACHE:
        _NC_CACHE["nc"] = build_nc()
    nc = _NC_CACHE["nc"]

    x = np.asarray(x, np.float32)
    Wqkv = np.asarray(Wqkv, np.float32)
    Wproj = np.asarray(Wproj, np.float32)
    cost, sint, trim_l, selm = _host_tables()
    idnm = np.eye(P, dtype=ml_dtypes.bfloat16)
    onbm = np.ones((P, P), ml_dtypes.bfloat16)

    Wq, Wk, Wv = Wqkv[:, 0:C], Wqkv[:, C : 2 * C], Wqkv[:, 2 * C : 3 * C]

    def wtile(Wm, hf):  # [C, 1024] -> [HL, P, NCT, P] bf16
        Wl = Wm[:, hf * CLOC : (hf + 1) * CLOC]
        return np.ascontiguousarray(
            Wl.reshape(NCT, P, HL, P).transpose(2, 1, 0, 3).astype(ml_dtypes.bfloat16)
        )

    in_maps = []
    for c in range(8):
        b, hf = c // 2, c % 2
        xTc = np.ascontiguousarray(
            x[b].T.reshape(NCT, P, T).transpose(1, 0, 2).astype(ml_dtypes.bfloat16)
        )  # [P, NCT, T]
        wvc = np.ascontiguousarray(
            Wv[:, hf * CLOC : (hf + 1) * CLOC]
            .reshape(NCT, P, CLOC)
            .transpose(1, 0, 2)
            .astype(ml_dtypes.bfloat16)
        )  # [P, NCT, CLOC]
        wpc = np.ascontiguousarray(
            Wproj[hf * CLOC : (hf + 1) * CLOC, :]
            .reshape(HL, P, C)
            .transpose(1, 0, 2)
            .astype(ml_dtypes.bfloat16)
        )  # [P, HL, C]
        in_maps.append(
            {
                "xT": xTc,
                "wq": wtile(Wq, hf),
                "wk": wtile(Wk, hf),
                "wv": wvc,
                "wp": wpc,
                "cost": cost,
                "sint": sint,
                "idnm": idnm,
                "onbm": onbm,
                "trim": trim_l,
                "selm": selm,
            }
        )

    if _PROFILE.get("on"):
        res = _run_timed(nc, in_maps)
    else:
        res = run_bass_kernel_spmd(nc, in_maps, core_ids=list(range(8)))
    out = np.empty((B, T, C), np.float32)
    for c in range(8):
        b, hf = c // 2, c % 2
        r = np.asarray(res.results[c]["out"], dtype=np.float32)  # [1024, T]
        for g in range(4):
            cout0 = g * 512 + hf * 256
            out[b, :, cout0 : cout0 + 256] = r[g * 256 : (g + 1) * 256].T
    return out


if __name__ == "__main__":
    nc = build_nc()
    print("graph built ok:", len(nc.m.functions[0].allocations), "allocations")
